# revision 21
# baseline (speedup 1.0000x reference)
"""Euclidean distance block (retrieval kNN) on 8 TRN2 NeuronCores.

dist[b, s, p] = sqrt(sum_c (x1[b, c, p] - x2[b, s, c, p])^2)   p = spatial (h*w)
out[b] = dist[b].reshape(S * h * w)

Sharding: data-parallel over batch B=32 -> 4 batches per core, no comms.

Per-core kernel layout: SBUF partitions carry (support_pair, channel) = 2*64 =
128; the free axis carries spatial. A big tile covers 8 supports as
[128, 4, 1764] in one fully-contiguous 3.6 MB DMA (f32 HBM -> bf16 SBUF cast
on the SWDGE ring). Compute chain per tile:
  DVE subtract in bf16 (2x mode), in place
  ACT Square -> bf16
  PE matmul against [128, 25] one-hot pair masks, accumulating per-support
    sums over C into a [25, 441] PSUM tile per spatial quarter
  ACT Sqrt PSUM -> SBUF f32, one contiguous 176 KB store per batch on the
    Scalar HWDGE ring (which never blocks loads).

DMA ring assignment matters: HWDGE rings execute FIFO per issuing engine, so
a store waiting on compute would stall every load queued behind it. Loads
(with cast) go on the GpSimd SWDGE ring, stores on Scalar, and the sync ring
only carries the mask load and the x1 partition-duplicate copy.
"""

import numpy as np

B, S, C, H, W = 32, 25, 64, 42, 42
HW = H * W            # 1764
NCORES = 8
BL = B // NCORES      # 4 batches per core
NSO = 4               # support pairs per big tile (8 supports)
NBIG = 3              # big tiles per batch (24 supports), then 1 leftover
NQ = 4                # spatial quarters
QW = HW // NQ         # 441
NPAIR = 13            # 12 support pairs + 1 leftover single

BF16_SUB = True       # bf16 inputs to the subtract (2x DVE); False = f32

_cache = {}


def _build_nc():
    import concourse.bacc as bacc
    import concourse.mybir as mybir
    from concourse.tile import TileContext
    from concourse.bass import MemorySpace

    f32 = mybir.dt.float32
    bf16 = mybir.dt.bfloat16
    ldt = bf16 if BF16_SUB else f32
    Square = mybir.ActivationFunctionType.Square
    Sqrt = mybir.ActivationFunctionType.Sqrt
    sub = mybir.AluOpType.subtract

    # Square and Sqrt both live in the "sqrt_and_others" act-function set,
    # but the table-load chooser picks the first set containing each one,
    # alternating two ~2.7us table reloads per batch. Strip the two
    # functions from every other set (contents only — set ids are
    # positional) so one resident table serves the whole kernel.
    _orig_tables = bacc.get_activation_tables

    def _pinned_tables(arch):
        t = _orig_tables(arch)
        for name, fns in t.items():
            if name != "sqrt_and_others":
                fns.discard(Square)
                fns.discard(Sqrt)
        return t

    bacc.get_activation_tables = _pinned_tables
    nc = bacc.Bacc()
    x1 = nc.declare_dram_parameter("x1", [BL, C, HW], f32, isOutput=False)
    x2 = nc.declare_dram_parameter("x2", [BL, S, C, HW], f32, isOutput=False)
    mk = nc.declare_dram_parameter("mask", [NPAIR, 128, S], bf16, isOutput=False)
    out = nc.declare_dram_parameter("out", [BL, S * HW], f32, isOutput=True)

    # loads: cast f32->ldt needs SWDGE (gpsimd); plain copies can go anywhere
    load = nc.gpsimd if BF16_SUB else nc.sync

    with TileContext(nc) as tc:
        with (
            tc.tile_pool(name="x2p", bufs=4) as x2p,
            tc.tile_pool(name="sqp", bufs=3) as sqp,
            tc.tile_pool(name="x1p", bufs=1) as x1p,
            tc.tile_pool(name="outp", bufs=2) as outp,
            tc.tile_pool(name="cst", bufs=1) as cst,
            tc.tile_pool(name="x2fp", bufs=1) as x2fp,
            tc.tile_pool(name="ps", bufs=2, space=MemorySpace.PSUM) as psp,
        ):
            mt = cst.tile([128, NPAIR, S], bf16)
            nc.sync.dma_start(mt[:], mk.rearrange("g k m -> k g m"))

            # all of x1 once: [c, b, p] on partitions 0..63, then duplicate
            # onto 64..127 via SBUF->SBUF (no extra HBM traffic)
            x1all = x1p.tile([128, BL, HW], ldt)
            load.dma_start(x1all[0:64, :, :], x1.rearrange("b c p -> c b p"))
            nc.sync.dma_start(x1all[64:128, :, :], x1all[0:64, :, :])

            # The first gpsimd (SWDGE) DMA pays ~6us of Q7 library-load +
            # descriptor warmup before the first HBM byte moves. Stream the
            # first batch's first tile (and its x1) in f32 over the sync
            # HWDGE ring instead, so HBM traffic starts immediately.
            x1f = None
            if BF16_SUB:
                x1f = cst.tile([128, HW], f32, name="x1f")
                nc.sync.dma_start(x1f[0:64, :], x1[0])
                nc.sync.dma_start(x1f[64:128, :], x1[0])

            for b in range(BL):
                # b=0 starts on the warm sync ring in f32; leftover-first
                # ordering only for b>0 (for b=0 the leftover data arrives
                # late, after the gpsimd warmup)
                leftover_first = b > 0 or not BF16_SUB

                # leftover support 24: DMA early so it streams with big tiles
                x2l = x2p.tile([64, HW], ldt, tag="x2l")
                load.dma_start(x2l[:], x2[b, S - 1])

                pst = [
                    psp.tile([S, QW], f32, name=f"ps{q}", tag=f"ps{q}")
                    for q in range(NQ)
                ]

                def leftover_compute(b=b, x2l=x2l, pst=pst, first=True):
                    # leftover compute first keeps the end-of-batch tail short
                    nc.vector.tensor_tensor(x2l[:], x2l[:], x1all[0:64, b, :], sub)
                    sql = sqp.tile([64, HW], bf16, name="sql", tag="sql")
                    nc.scalar.activation(sql[:], x2l[:], Square)
                    for q in range(NQ):
                        nc.tensor.matmul(
                            pst[q][:, :],
                            mt[0:64, NPAIR - 1, :],
                            sql[:, q * QW : (q + 1) * QW],
                            start=first,
                            stop=not first,
                        )

                if leftover_first:
                    leftover_compute(first=True)

                for i in range(NBIG):
                    f32_tile = BF16_SUB and b == 0 and i == 0
                    if f32_tile:
                        x2t = x2fp.tile([128, NSO, HW], f32, name="x2tf", tag="x2tf")
                        x1s = x1f[:, :]
                        ring = nc.sync
                    else:
                        x2t = x2p.tile([128, NSO, HW], ldt, tag="x2t")
                        x1s = x1all[:, b, :]
                        ring = load
                    src = x2[b, 8 * i : 8 * i + 8].rearrange(
                        "(so si) c p -> (si c) so p", si=2
                    )
                    # per-pair DMAs: same streaming rate, but 4x finer
                    # completion sems -> subs start on the first 902KB
                    for so in range(NSO):
                        ring.dma_start(x2t[:, so, :], src[:, so, :])
                    sq = sqp.tile([128, NSO, HW], bf16, tag="sq")
                    for so in range(NSO):
                        # in-place: x2t slice becomes diff
                        nc.vector.tensor_tensor(x2t[:, so, :], x2t[:, so, :], x1s, sub)
                        # squares split 3/1 across ACT and DVE to balance
                        # the per-tile engine cadence
                        if so < 3:
                            nc.scalar.activation(sq[:, so, :], x2t[:, so, :], Square)
                        else:
                            nc.vector.tensor_tensor(
                                sq[:, so, :],
                                x2t[:, so, :],
                                x2t[:, so, :],
                                mybir.AluOpType.mult,
                            )
                        j = NSO * i + so
                        for q in range(NQ):
                            nc.tensor.matmul(
                                pst[q][:, :],
                                mt[:, j, :],
                                sq[:, so, q * QW : (q + 1) * QW],
                                start=(j == 0 and not leftover_first),
                                stop=(j == NPAIR - 2 and leftover_first),
                            )

                if not leftover_first:
                    leftover_compute(first=False)

                ot = outp.tile([S, HW], f32, tag="ot")
                for q in range(NQ):
                    # per-quarter sqrt + store: each store issues as soon as
                    # its quarter's accumulation closes
                    nc.scalar.activation(ot[:, q * QW : (q + 1) * QW], pst[q][:], Sqrt)
                    # store via the Scalar HWDGE ring: ACT reaches this only
                    # after its own sqrt, so the wait is pre-satisfied; a
                    # store on a load ring would stall loads queued behind it
                    nc.scalar.dma_start(
                        out[b].rearrange("(s p) -> s p", s=S)[:, q * QW : (q + 1) * QW],
                        ot[:, q * QW : (q + 1) * QW],
                    )

    try:
        nc.finalize()
    finally:
        bacc.get_activation_tables = _orig_tables
    return nc


def get_nc():
    if "nc" not in _cache:
        _cache["nc"] = _build_nc()
    return _cache["nc"]


def make_mask() -> np.ndarray:
    # mask[j, k, m] = 1 iff partition k of pair-tile j feeds output support m.
    # Pair j < 12 covers supports (2j, 2j+1): k < 64 -> 2j, k >= 64 -> 2j+1.
    # Pair 12 is the leftover single support 24 on partitions 0..63.
    import ml_dtypes

    mask = np.zeros((NPAIR, 128, S), dtype=ml_dtypes.bfloat16)
    for j in range(NPAIR - 1):
        mask[j, 0:64, 2 * j] = 1.0
        mask[j, 64:128, 2 * j + 1] = 1.0
    mask[NPAIR - 1, 0:64, S - 1] = 1.0
    return mask


def make_in_maps(x1: np.ndarray, x2: np.ndarray) -> list[dict]:
    x1 = np.ascontiguousarray(np.asarray(x1, dtype=np.float32)).reshape(B, C, HW)
    x2 = np.ascontiguousarray(np.asarray(x2, dtype=np.float32)).reshape(B, S, C, HW)
    mask = make_mask()
    maps = []
    for i in range(NCORES):
        sl = slice(i * BL, (i + 1) * BL)
        maps.append({"x1": x1[sl], "x2": x2[sl], "mask": mask})
    return maps


def gather_out(results: list[dict]) -> np.ndarray:
    return np.concatenate([np.asarray(r["out"]) for r in results], axis=0).astype(
        np.float32, copy=False
    )


def kernel(x1, x2) -> np.ndarray:
    from concourse.bass_utils import run_bass_kernel_spmd

    nc = get_nc()
    in_maps = make_in_maps(x1, x2)
    res = run_bass_kernel_spmd(nc, in_maps, list(range(NCORES)))
    return gather_out(res.results)
